# revision 14
# baseline (speedup 1.0000x reference)
"""Fused DropBlock_Ske + DropBlockT_1d kernel for Trainium2 (8 NeuronCores).

The reference nn.Module's coordinate-attention branch is dead code w.r.t. the
output, which reduces to

    out[n,c,t,v] = x[n,c,t,v] * mk_s[n,v] * mk_t[n,t] * scale

where mk_s/mk_t are 0/1 masks derived from tiny inputs (mask_s, mask_t, u_s,
u_t, A) and scale is a global scalar.  The mask math is O(NM*(V+T)) and is
done on host; the device kernel performs the single memory-bound pass over x,
data-parallel over the batch dim (8 batches per core).

I/O rides in fp16 (quantize on host, widen after), halving HBM traffic -- the
sole roofline -- at a ~4e-4 relative-error cost vs the 2e-2 gate.

Per batch the kernel builds the combined mask comb[p,tv] = mk_t[t]*mv_eff[v]
(3200 elements, partition-replicated) and multiplies the batch tile by it.
The big multiply is an all-fp16 packed SBUF tensor_tensor, which runs in the
DVE 2x perf mode.  The comb outer product has a stride-0 innermost operand,
which forces 1x -- so half the builds go to the otherwise-idle GPSIMD (Pool)
engine, keeping both engines well below the ~65us DMA floor.  The tiny mask
tile loads on the ACT-engine DGE ring at t~1us so comb building never waits
on the big x loads.
"""

import numpy as np

NM, C, T, V = 64, 256, 128, 25
N_CORES = 8
NPC = NM // N_CORES          # batches per core
TV = T * V                   # 3200
P = 128                      # SBUF partitions
MASK_COLS = NPC * (V + T)    # 1224

KEEP_PROB = 0.9
BLOCK_SIZE = 7

# Set by test harness only: trace the run and stash results for profiling.
TRACE = False
LAST_RESULT = None

_BASS = {"nc": None}


def _compute_masks(A, mask_s, mask_t, u_s, u_t):
    """Replicates the reference's mask math in float32 numpy.

    Returns mv_eff (NM, V) = mk_s * combined_scale and mk_t (NM, T)."""
    f32 = np.float32
    A = np.asarray(A, f32)
    mask_s = np.asarray(mask_s, f32)
    mask_t = np.asarray(mask_t, f32)
    u_s = np.asarray(u_s, f32)
    u_t = np.asarray(u_t, f32).reshape(NM, T)

    # ---- DropBlock_Ske ----
    gamma_s = f32((1.0 - KEEP_PROB) / (1.0 + 1.92))
    ms = mask_s / mask_s.sum() * f32(mask_s.size)
    p_s = np.minimum(ms * gamma_s, f32(1.0))
    m_seed = (u_s < p_s).astype(f32)
    m = ((m_seed @ A) > f32(0.001)).astype(f32)
    mk_s = f32(1.0) - m                                   # (NM, V), 0/1
    scale_s = float(NM * V) / max(float(mk_s.sum()), 1.0)

    # ---- DropBlockT_1d ----
    gamma_t = f32((1.0 - KEEP_PROB) / BLOCK_SIZE)
    mt = mask_t / mask_t.sum() * f32(mask_t.size)
    p_t = np.minimum(mt * gamma_t, f32(1.0))
    m_t = (u_t < p_t).astype(f32)                         # (NM, T), 0/1
    pad = BLOCK_SIZE // 2
    mp = np.pad(m_t, ((0, 0), (pad, pad)), constant_values=0.0)
    msum = m_t.copy()
    for i in range(BLOCK_SIZE):
        np.maximum(msum, mp[:, i:i + T], out=msum)
    mk_t = f32(1.0) - msum                                # (NM, T), 0/1
    numel = float(NM * C * T * V)
    scale_t = numel / max(float(mk_t.sum()) * (C * V), 1.0)

    mv_eff = mk_s * f32(scale_s * scale_t)
    return mv_eff.astype(f32), mk_t.astype(f32)


def _build_bass():
    import concourse.bass as bass
    import concourse.mybir as mybir
    from concourse.tile import TileContext, add_dep_helper

    f16 = mybir.dt.float16
    nc = bass.Bass()
    xs = nc.dram_tensor("xs", [NPC * P, 2 * TV], f16, kind="ExternalInput")
    # mv_eff | mk_t blocks, host-replicated across all 128 partitions
    mrow = nc.dram_tensor("mrow", [P, MASK_COLS], f16, kind="ExternalInput")
    out = nc.dram_tensor("out", [NPC * P, 2 * TV], f16,
                         kind="ExternalOutput")
    MT_OFF = NPC * V

    # Every TPB instruction (compute AND DMA) has exactly ONE sync-wait
    # slot, and sync-wait elision is strictly per-proc.  Structure:
    #  - 8 HWDGE x loads and 8 SWDGE stores use each lane sem once; the
    #    tiny mask load rides the ACT-engine DGE ring;
    #  - comb builds wait only the mask-ring sem (elided after the first
    #    per engine);
    #  - per batch, 1-element DVE read-carriers absorb the load-lane wait
    #    (and for Pool-built combs the Pool wait), so the big multiply
    #    needs only one self-engine wait;
    #  - a gpsimd memset with a forced sync dep absorbs each load's lane
    #    wait on the store ring, so each store needs only its DVE wait;
    #  - no-sync scheduler edges pin per-engine tick order.
    with TileContext(nc) as tc:
        with tc.tile_pool(name="comb", bufs=NPC) as combpool, \
             tc.tile_pool(name="mrowp", bufs=1) as mrowpool, \
             tc.tile_pool(name="scratch", bufs=NPC) as spool, \
             tc.tile_pool(name="ascratch", bufs=NPC) as apool, \
             tc.tile_pool(name="pscr", bufs=NPC) as ppool, \
             tc.tile_pool(name="work", bufs=NPC) as pool:
            ns = lambda a, b: add_dep_helper(a.ins, b.ins, sync=False,
                                             reason="tick ordering")
            mrow_sb = mrowpool.tile([P, MASK_COLS], f16, tag="mrow")
            mrow_ld = nc.scalar.dma_start(mrow_sb[:, :], mrow[:, :])

            tiles, loads = [], []
            for i in range(NPC):
                t = pool.tile([P, 2 * TV], f16)
                ld = nc.sync.dma_start(t[:, :], xs[i * P:(i + 1) * P, :])
                if i >= 1:
                    ns(ld, loads[-1])
                tiles.append(t); loads.append(ld)

            # comb[i][p, t*V+v] = mk_t[i,t] * mv_eff[i,v]; identical on
            # every partition.  Even batches build on DVE, odd on Pool.
            combs, cbs = [], []
            dve_prev = pool_prev = None
            for i in range(NPC):
                comb = combpool.tile([P, TV], f16)
                mv_b = mrow_sb[:, i * V:(i + 1) * V] \
                    .unsqueeze(1).broadcast_to([P, T, V])
                mt_b = mrow_sb[:, MT_OFF + i * T:MT_OFF + (i + 1) * T] \
                    .unsqueeze(2).broadcast_to([P, T, V])
                comb3 = comb[:, :].rearrange("p (t v) -> p t v", v=V)
                eng = nc.vector if i % 2 == 0 else nc.gpsimd
                cb = eng.tensor_tensor(out=comb3, in0=mt_b, in1=mv_b,
                                       op=mybir.AluOpType.mult)
                if i % 2 == 0:
                    if dve_prev is not None:
                        ns(cb, dve_prev)
                    dve_prev = cb
                else:
                    if pool_prev is not None:
                        ns(cb, pool_prev)
                    pool_prev = cb
                combs.append(comb); cbs.append(cb)

            applies, stores = [], []
            for i in range(NPC):
                t = tiles[i]
                comb = combs[i]
                scratch = spool.tile([P, 1], f16)
                # read-carrier: sole absorber of the load-lane wait on DVE.
                tcar = nc.vector.tensor_tensor(out=scratch[:, :],
                                               in0=t[:, 1:2], in1=t[:, 1:2],
                                               op=mybir.AluOpType.mult)
                ns(tcar, cbs[i])
                last_car = tcar
                if i % 2 == 1:
                    # second carrier absorbs the Pool sem for the
                    # Pool-built comb.
                    ascratch = apool.tile([P, 1], f16)
                    acar = nc.vector.tensor_tensor(
                        out=ascratch[:, :],
                        in0=comb[:, TV - 1:TV], in1=comb[:, TV - 1:TV],
                        op=mybir.AluOpType.mult)
                    ns(acar, tcar)
                    last_car = acar
                # Fused apply over both channel halves: comb broadcasts
                # along the 2-wide middle dim.  All operands fp16, packed,
                # SBUF -> DVE 2x perf mode.
                t3 = t[:, 0:2 * TV].rearrange("p (h tv) -> p h tv", h=2)
                comb_b = comb[:, :].unsqueeze(1).broadcast_to([P, 2, TV])
                ap = nc.vector.tensor_tensor(out=t3, in0=t3, in1=comb_b,
                                             op=mybir.AluOpType.mult)
                ns(ap, last_car)
                # store-ring lane absorber: a write-only no-op with a
                # forced sync dep on the load; it carries the load-lane
                # wait so the store (whose writer list still includes the
                # load) needs only its DVE wait.
                pscr = ppool.tile([1, 1], f16)
                pcar = nc.gpsimd.memset(pscr[0:1, 0:1], 0.0)
                add_dep_helper(pcar.ins, loads[i].ins, sync=True,
                               reason="ring lane absorber")
                st = nc.gpsimd.dma_start(out[i * P:(i + 1) * P, :],
                                         t[:, 0:2 * TV])
                ns(st, pcar)
                if i >= 1:
                    ns(tcar, applies[-1])
                    ns(st, stores[-1])
                applies.append(ap); stores.append(st)

            # Tail: absorb each outstanding sem into the SP sequencer's
            # observed set with a chain of 1-wait nops so the framework
            # drain needs no multi-wait instruction.
            ptail = nc.gpsimd.memset(pscr[0:1, 0:1], 0.0)
            add_dep_helper(ptail.ins, stores[-1].ins, sync=False,
                           reason="final pool op")
            prev = None
            tail_deps = list(stores) + list(loads) + \
                [mrow_ld, applies[-1], ptail]
            for dep in tail_deps:
                nop = nc.sync.nop()
                add_dep_helper(nop.ins, dep.ins, sync=True,
                               reason="drain pre-absorb")
                add_dep_helper(nop.ins,
                               (prev if prev is not None else loads[-1]).ins,
                               sync=False, reason="tail order")
                prev = nop
    return nc


def kernel(x, A, mask_s, mask_t, u_s, u_t, w1, b1, bn_gamma, bn_beta,
           wh, bh, ww, bw):
    global LAST_RESULT
    from concourse.bass_utils import run_bass_kernel_spmd

    f16 = np.float16
    x = np.asarray(x, np.float32).astype(f16)
    mv_eff, mk_t = _compute_masks(A, mask_s, mask_t, u_s, u_t)
    mv_eff = mv_eff.astype(f16)
    mk_t = mk_t.astype(f16)

    in_maps = []
    for k in range(N_CORES):
        sl = slice(k * NPC, (k + 1) * NPC)
        mask_row = np.concatenate(
            [mv_eff[sl].reshape(NPC * V), mk_t[sl].reshape(NPC * T)])
        mrow = np.broadcast_to(mask_row[None, :], (P, MASK_COLS)).copy()
        xk = np.ascontiguousarray(x[sl].reshape(NPC * P, 2 * TV))
        in_maps.append({"xs": xk, "mrow": mrow})

    if _BASS["nc"] is None:
        _BASS["nc"] = _build_bass()

    res = run_bass_kernel_spmd(_BASS["nc"], in_maps, list(range(N_CORES)),
                               trace=TRACE)
    LAST_RESULT = res

    out = np.empty((NM, C, T, V), np.float32)
    for k in range(N_CORES):
        out[k * NPC:(k + 1) * NPC] = \
            res.results[k]["out"].reshape(NPC, C, T, V).astype(np.float32)
    return out


# revision 16
# speedup vs baseline: 1.2356x; 1.2356x over previous
"""Fused DropBlock_Ske + DropBlockT_1d kernel for Trainium2 (8 NeuronCores).

The reference nn.Module's coordinate-attention branch is dead code w.r.t. the
output, which reduces to

    out[n,c,t,v] = x[n,c,t,v] * mk_s[n,v] * mk_t[n,t] * scale

where mk_s/mk_t are 0/1 masks derived from tiny inputs (mask_s, mask_t, u_s,
u_t, A) and scale is a global scalar.  The mask math is O(NM*(V+T)) and is
done on host; the device kernel performs the single memory-bound pass over x,
data-parallel over the batch dim (8 batches per core).

I/O rides in fp16 (quantize on host, widen after), halving HBM traffic -- the
sole roofline -- at a ~4e-4 relative-error cost vs the 2e-2 gate.

The host reorders x to [n, t, c, v] so each batch tile has T=128 on the
partition dim.  Then the per-batch combined mask [t, v] is ONE tensor_scalar
op (mv replicated along free, mk_t[t] as a per-partition scalar; DVE 4x perf
mode, ~0.1us), and the big multiply is ONE all-fp16 packed SBUF tensor_tensor
(DVE 2x mode, ~3.5us) with the mask broadcast along the C middle dim.  DVE
work per batch (~3.75us) undercuts the ~4.1us DMA load cadence, so applies
and stores chase the loads and the kernel is DMA-bound end to end.
"""

import numpy as np

NM, C, T, V = 64, 256, 128, 25
N_CORES = 8
NPC = NM // N_CORES          # batches per core
CV = C * V                   # 6400 free elements per (batch, t)
P = 128                      # SBUF partitions == T

KEEP_PROB = 0.9
BLOCK_SIZE = 7

# Set by test harness only: trace the run and stash results for profiling.
TRACE = False
LAST_RESULT = None

_BASS = {"nc": None}


def _compute_masks(A, mask_s, mask_t, u_s, u_t):
    """Replicates the reference's mask math in float32 numpy.

    Returns mv_eff (NM, V) = mk_s * combined_scale and mk_t (NM, T)."""
    f32 = np.float32
    A = np.asarray(A, f32)
    mask_s = np.asarray(mask_s, f32)
    mask_t = np.asarray(mask_t, f32)
    u_s = np.asarray(u_s, f32)
    u_t = np.asarray(u_t, f32).reshape(NM, T)

    # ---- DropBlock_Ske ----
    gamma_s = f32((1.0 - KEEP_PROB) / (1.0 + 1.92))
    ms = mask_s / mask_s.sum() * f32(mask_s.size)
    p_s = np.minimum(ms * gamma_s, f32(1.0))
    m_seed = (u_s < p_s).astype(f32)
    m = ((m_seed @ A) > f32(0.001)).astype(f32)
    mk_s = f32(1.0) - m                                   # (NM, V), 0/1
    scale_s = float(NM * V) / max(float(mk_s.sum()), 1.0)

    # ---- DropBlockT_1d ----
    gamma_t = f32((1.0 - KEEP_PROB) / BLOCK_SIZE)
    mt = mask_t / mask_t.sum() * f32(mask_t.size)
    p_t = np.minimum(mt * gamma_t, f32(1.0))
    m_t = (u_t < p_t).astype(f32)                         # (NM, T), 0/1
    pad = BLOCK_SIZE // 2
    mp = np.pad(m_t, ((0, 0), (pad, pad)), constant_values=0.0)
    msum = m_t.copy()
    for i in range(BLOCK_SIZE):
        np.maximum(msum, mp[:, i:i + T], out=msum)
    mk_t = f32(1.0) - msum                                # (NM, T), 0/1
    numel = float(NM * C * T * V)
    scale_t = numel / max(float(mk_t.sum()) * (C * V), 1.0)

    mv_eff = mk_s * f32(scale_s * scale_t)
    return mv_eff.astype(f32), mk_t.astype(f32)


MROW_COLS = NPC * V + NPC    # per-batch mv blocks, then per-batch mt column


def _build_bass():
    import concourse.bass as bass
    import concourse.mybir as mybir
    from concourse.tile import TileContext, add_dep_helper

    f16 = mybir.dt.float16
    f32 = mybir.dt.float32
    nc = bass.Bass()
    # x reordered host-side to [n, t, c, v]; tile i = batch i as
    # (128 partitions = t, C*V free).
    xs = nc.dram_tensor("xs", [NPC * P, CV], f16, kind="ExternalInput")
    # cols [i*V:(i+1)*V] = mv_eff[i] replicated across partitions;
    # col NPC*V + i = mk_t[i, t] at partition t (per-partition scalars).
    mrow = nc.dram_tensor("mrow", [P, MROW_COLS], f16, kind="ExternalInput")
    out = nc.dram_tensor("out", [NPC * P, CV], f16, kind="ExternalOutput")
    MT_OFF = NPC * V

    # Every TPB instruction (compute AND DMA) has exactly ONE sync-wait
    # slot, and sync-wait elision is strictly per-proc.  Structure:
    #  - 8 HWDGE x loads and 8 SWDGE stores use each lane sem once; the
    #    tiny mask load rides the ACT-engine DGE ring, and a 1-element DVE
    #    converter op absorbs its lane wait (and widens mk_t to the f32
    #    the tensor_scalar per-partition scalar requires);
    #  - per batch: comb = tensor_scalar (DVE 4x), a 1-element
    #    read-carrier absorbs the load-lane wait, and the fused multiply
    #    (DVE 2x) then needs only one self-engine wait;
    #  - a gpsimd memset with a forced sync dep absorbs each load's lane
    #    wait on the store ring, so each store needs only its DVE wait;
    #  - no-sync scheduler edges pin per-engine tick order.
    with TileContext(nc) as tc:
        with tc.tile_pool(name="comb", bufs=NPC) as combpool, \
             tc.tile_pool(name="mrowp", bufs=1) as mrowpool, \
             tc.tile_pool(name="scratch", bufs=NPC) as spool, \
             tc.tile_pool(name="pscr", bufs=NPC) as ppool, \
             tc.tile_pool(name="work", bufs=NPC) as pool:
            ns = lambda a, b: add_dep_helper(a.ins, b.ins, sync=False,
                                             reason="tick ordering")
            mrow_sb = mrowpool.tile([P, MROW_COLS], f16, tag="mrow")
            mtc32 = mrowpool.tile([P, NPC], f32, tag="mtc32")
            mrow_ld = nc.scalar.dma_start(mrow_sb[:, :], mrow[:, :])
            # fp16 -> f32 widen of the mk_t columns; also the sole
            # absorber of the mask load's ring-lane wait on the DVE.
            conv = nc.vector.tensor_scalar(
                out=mtc32[:, :], in0=mrow_sb[:, MT_OFF:MT_OFF + NPC],
                scalar1=1.0, scalar2=None, op0=mybir.AluOpType.mult)

            tiles, loads = [], []
            for i in range(NPC):
                t = pool.tile([P, CV], f16)
                ld = nc.sync.dma_start(t[:, :], xs[i * P:(i + 1) * P, :])
                if i >= 1:
                    ns(ld, loads[-1])
                tiles.append(t); loads.append(ld)

            applies, stores = [], []
            prev_dve = conv
            for i in range(NPC):
                t = tiles[i]
                # comb[t, v] = mv_eff[v] * mk_t[t]: tensor_scalar with a
                # per-partition scalar -- DVE 4x mode, ~0.1us.
                comb = combpool.tile([P, V], f16)
                cb = nc.vector.tensor_scalar(
                    out=comb[:, :], in0=mrow_sb[:, i * V:(i + 1) * V],
                    scalar1=mtc32[:, i:i + 1], scalar2=None,
                    op0=mybir.AluOpType.mult)
                ns(cb, prev_dve)
                scratch = spool.tile([P, 1], f16)
                # read-carrier: sole absorber of the load-lane wait.
                tcar = nc.vector.tensor_tensor(out=scratch[:, :],
                                               in0=t[:, 1:2], in1=t[:, 1:2],
                                               op=mybir.AluOpType.mult)
                ns(tcar, cb)
                # Fused apply: comb broadcasts along the C middle dim.
                # All operands fp16, packed, SBUF -> DVE 2x perf mode.
                t3 = t[:, :].rearrange("p (c v) -> p c v", v=V)
                comb_b = comb[:, :].unsqueeze(1).broadcast_to([P, C, V])
                ap = nc.vector.tensor_tensor(out=t3, in0=t3, in1=comb_b,
                                             op=mybir.AluOpType.mult)
                ns(ap, tcar)
                prev_dve = ap
                # store-ring lane absorber: a write-only no-op with a
                # forced sync dep on the load; it carries the load-lane
                # wait so the store (whose writer list still includes the
                # load) needs only its DVE wait.
                pscr = ppool.tile([1, 1], f16)
                pcar = nc.gpsimd.memset(pscr[0:1, 0:1], 0.0)
                add_dep_helper(pcar.ins, loads[i].ins, sync=True,
                               reason="ring lane absorber")
                st = nc.gpsimd.dma_start(out[i * P:(i + 1) * P, :],
                                         t[:, :])
                ns(st, pcar)
                if i >= 1:
                    ns(st, stores[-1])
                applies.append(ap); stores.append(st)

            # Tail: absorb each outstanding sem into the SP sequencer's
            # observed set with a chain of 1-wait nops so the framework
            # drain needs no multi-wait instruction.
            ptail = nc.gpsimd.memset(pscr[0:1, 0:1], 0.0)
            add_dep_helper(ptail.ins, stores[-1].ins, sync=False,
                           reason="final pool op")
            prev = None
            tail_deps = list(stores) + list(loads) + \
                [mrow_ld, applies[-1], ptail]
            for dep in tail_deps:
                nop = nc.sync.nop()
                add_dep_helper(nop.ins, dep.ins, sync=True,
                               reason="drain pre-absorb")
                add_dep_helper(nop.ins,
                               (prev if prev is not None else loads[-1]).ins,
                               sync=False, reason="tail order")
                prev = nop
    return nc


def kernel(x, A, mask_s, mask_t, u_s, u_t, w1, b1, bn_gamma, bn_beta,
           wh, bh, ww, bw):
    global LAST_RESULT
    from concourse.bass_utils import run_bass_kernel_spmd

    f16 = np.float16
    # reorder to [n, t, c, v] so T rides the partition dim on device
    xt = np.ascontiguousarray(
        np.asarray(x, np.float32).transpose(0, 2, 1, 3)).astype(f16)
    mv_eff, mk_t = _compute_masks(A, mask_s, mask_t, u_s, u_t)

    in_maps = []
    for k in range(N_CORES):
        sl = slice(k * NPC, (k + 1) * NPC)
        mrow = np.empty((P, MROW_COLS), f16)
        mrow[:, :NPC * V] = np.broadcast_to(
            mv_eff[sl].reshape(1, NPC * V), (P, NPC * V)).astype(f16)
        mrow[:, NPC * V:] = mk_t[sl].T.astype(f16)   # [T, NPC], 0/1 exact
        xk = xt[sl].reshape(NPC * P, CV)
        in_maps.append({"xs": xk, "mrow": mrow})

    if _BASS["nc"] is None:
        _BASS["nc"] = _build_bass()

    res = run_bass_kernel_spmd(_BASS["nc"], in_maps, list(range(N_CORES)),
                               trace=TRACE)
    LAST_RESULT = res

    out = np.empty((NM, C, T, V), np.float32)
    for k in range(N_CORES):
        out[k * NPC:(k + 1) * NPC] = res.results[k]["out"] \
            .reshape(NPC, T, C, V).transpose(0, 2, 1, 3).astype(np.float32)
    return out
